# revision 40
# baseline (speedup 1.0000x reference)
"""Trainium2 Bass kernel for nn_Attention_3315714753146 (gnn_message_passing).

out = (LA*softmax(mask(QK^T*scale)) + LG*adj_masked + LD*exp(-dist_masked)) @ V @ W_out + b_out

Sharding: 8 shards = (4 batches) x (2 query-row halves of 512 rows). Each core
computes its own 512 output rows from full K/V (computed on-device from x).

Key structure (per core):
  - Softmax path in fp8: q/k projections via fp8 DoubleRow matmuls (weights
    pre-scaled x16 on host), dots as fp8 matmuls, p = exp(dots*scale/256) in
    fp8 ([128,1024] two-bank ACT reads), PV via fp8 DoubleRow with an
    augmented mj/LA column computing the softmax denominator. Softmax term is
    ~0.3% of output norm, so fp8 noise here is invisible at the 2e-2 gate.
  - C0 = LG*adj + LD*exp(-dist) path in bf16: host supplies TRANSPOSED
    masked adjacency (fp8, exact 0/1) and distance (bf16) so no PE transposes
    are needed; cv = V^T @ C0T in bf16 (precision-dominant path).
  - b_out and the invalid-query-row correction (LA/N * colsum(V) @ W_out)
    enter as one K=2 rank-2 matmul per output row-chunk.
  - Emission is software-pipelined across head-pairs to keep the PE stream
    dense (HAM stays warm) and start the Scalar exp stream by ~4us.
"""

import sys

for _p in ("/root/.axon_site", "/root/.axon_site/_ro/trn_rl_repo",
           "/root/.axon_site/_ro/pypackages"):
    if _p not in sys.path:
        sys.path.append(_p)

import numpy as np
import ml_dtypes

BF = ml_dtypes.bfloat16
F8 = ml_dtypes.float8_e4m3
HEADS, DH = 8, 64
B, N, D = 4, 1024, 512
NH = 512          # query rows per core
LA = LD = LG = 0.33
SCALE = DH ** -0.5
NEG = -1e30
NCORES = 8
WSC = 16.0        # host prescale on wq/wk; dots come out x256
Q8 = float(np.float32(F8(1.0 / LA)))   # fp8 value used in denominator column
CORR = LA * Q8
# Schraudolph exp constants: exp(z) ~ bitcast_f32(int32(A*z + Bc))
SCH_A = 8388608.0 / float(np.log(2.0))
SCH_B = 127.0 * 8388608.0 - 366000.0
# which (hp, side, t) exp tiles run on DVE instead of Scalar
SCHRAUD = set()

_CACHE = {}


def _build_nc():
    import concourse.bass as bass
    import concourse.bacc as bacc
    import concourse.tile as tile
    from concourse import mybir
    from concourse.bass import ts

    F32 = mybir.dt.float32
    BF16 = mybir.dt.bfloat16
    F8D = mybir.dt.float8e4
    I32 = mybir.dt.int32
    AF = mybir.ActivationFunctionType
    OP = mybir.AluOpType
    DR = mybir.MatmulPerfMode.DoubleRow

    nc = bacc.Bacc()
    xT = nc.declare_dram_parameter("xT", [128, 4, N], BF16, isOutput=False)
    xq8 = nc.declare_dram_parameter("xq8", [128, 4, NH], F8D, isOutput=False)
    xk8 = nc.declare_dram_parameter("xk8", [128, 4, N], F8D, isOutput=False)
    wq8 = nc.declare_dram_parameter("wq8", [128, 4, D], F8D, isOutput=False)
    wk8 = nc.declare_dram_parameter("wk8", [128, 4, D], F8D, isOutput=False)
    wv = nc.declare_dram_parameter("wv", [D, D], BF16, isOutput=False)
    wout = nc.declare_dram_parameter("wout", [128, 4, D], BF16, isOutput=False)
    adjT8 = nc.declare_dram_parameter("adjT8", [N, NH], F8D, isOutput=False)
    distT = nc.declare_dram_parameter("distT", [N, NH], BF16, isOutput=False)
    # cvec cols: [0:8]=lnLD+ln(mj) per j-chunk, [8:16]=LG*mj, [16:24]=mj
    cvec = nc.declare_dram_parameter("cvec", [128, 24], F32, isOutput=False)
    mi2 = nc.declare_dram_parameter("mi2", [2, NH], F32, isOutput=False)
    mjq8 = nc.declare_dram_parameter("mjq8", [128, 8, 8], F8D, isOutput=False)
    bias2 = nc.declare_dram_parameter("bias2", [2, NH + 128], BF16, isOutput=False)
    b2r = nc.declare_dram_parameter("b2r", [2, D], BF16, isOutput=False)
    out = nc.declare_dram_parameter("out", [NH, D], BF16, isOutput=True)

    with tile.TileContext(nc) as tc:
        with (
            tc.tile_pool(name="const", bufs=1) as constp,
            tc.tile_pool(name="pers", bufs=1) as pers,
            tc.tile_pool(name="stage", bufs=2) as stagep,
            tc.tile_pool(name="p2p", bufs=16) as p2p,
            tc.tile_pool(name="sml", bufs=4) as smlp,
            tc.tile_pool(name="i32p", bufs=2) as i32p,
            tc.tile_pool(name="outp", bufs=2) as outp,
            tc.tile_pool(name="dpsp", bufs=5, space="PSUM") as dpsp,
            tc.tile_pool(name="pvp", bufs=1, space="PSUM") as pvp,
            tc.tile_pool(name="cvp", bufs=2, space="PSUM") as cvp,
        ):
            # ---------------- DMAs (priority order: q/k first) -------------
            xq8_sb = pers.tile([128, 4, NH], F8D, name="xq8_sb")
            nc.sync.dma_start(xq8_sb[:], xq8[:])
            wq8_sb = pers.tile([128, 4, D], F8D, name="wq8_sb")
            nc.sync.dma_start(wq8_sb[:], wq8[:])
            wk8_sb = pers.tile([128, 4, D], F8D, name="wk8_sb")
            nc.sync.dma_start(wk8_sb[:], wk8[:])
            xk8_sb = pers.tile([128, 4, N], F8D, name="xk8_sb")
            nc.sync.dma_start(xk8_sb[:], xk8[:])
            wv_sb = pers.tile([128, 4, D], BF16, name="wv_sb")
            for cc in range(4):
                nc.sync.dma_start(wv_sb[:, cc, :], wv[ts(cc, 128), :])
            xt = pers.tile([128, 4, N], BF16, name="xt")
            for ncc in range(8):
                nc.sync.dma_start(xt[:, :, ts(ncc, 128)], xT[:, :, ts(ncc, 128)])
            adj_sb = pers.tile([128, 8, NH], F8D, name="adj_sb")
            dist_sb = pers.tile([128, 8, NH], BF16, name="dist_sb")
            for jc in range(8):
                nc.sync.dma_start(dist_sb[:, jc, :], distT[ts(jc, 128), :])
            for jc in range(8):
                nc.sync.dma_start(adj_sb[:, jc, :], adjT8[ts(jc, 128), :])
            wout_sb = pers.tile([128, 4, D], BF16, name="wout_sb")
            nc.sync.dma_start(wout_sb[:], wout[:])
            cv_c = constp.tile([128, 24], F32, name="cv_c")
            nc.gpsimd.dma_start(cv_c[:], cvec[:])
            mi2_sb = constp.tile([2, NH], F32, name="mi2_sb")
            nc.gpsimd.dma_start(mi2_sb[:], mi2[:])
            bias2_sb = constp.tile([2, NH + 128], BF16, name="bias2_sb")
            nc.gpsimd.dma_start(bias2_sb[:], bias2[:])
            b2r_sb = constp.tile([2, D], BF16, name="b2r_sb")
            nc.gpsimd.dma_start(b2r_sb[:], b2r[:])
            mjq_sb = constp.tile([128, 8, 8], F8D, name="mjq_sb")
            nc.gpsimd.dma_start(mjq_sb[:], mjq8[:])

            # vau2[t]: [128, o, 8 heads * 80] fp8; cols h*80..h*80+63 = v*mj,
            # col h*80+64 = mj/LA, rest padding
            vau2 = [pers.tile([128, 2, 640], F8D, name=f"vau2_{t}") for t in range(4)]
            for t in range(4):
                for o in range(2):
                    v3 = vau2[t][:, o, :].rearrange("p (h e) -> p h e", e=80)
                    nc.gpsimd.tensor_copy(v3[:, :, 64], mjq_sb[:, 2 * t + o, :])

            v16 = pers.tile([128, 8, NH], BF16, name="v16")
            qt_sb = pers.tile([128, 4, NH], F8D, name="qt_sb")
            kt_sb = pers.tile([128, 4, N], F8D, name="kt_sb")
            c0T = pers.tile([128, 8, NH], BF16, name="c0T")
            itf_all = pers.tile([128, 4, NH], BF16, name="itf_all")

            def dps_tile():
                return dpsp.tile([128, 512], F32, name="dps", tag="dps")

            def emit_v(pr):
                for half in range(2):
                    ncc = 2 * pr + half
                    ps = dps_tile()
                    for cc in range(4):
                        nc.tensor.matmul(ps[:], lhsT=xt[:, cc, ts(ncc, 128)],
                                         rhs=wv_sb[:, cc, :], start=(cc == 0),
                                         stop=(cc == 3))
                    nc.vector.tensor_copy(v16[:, ncc, :], ps[:])
                    t, o = ncc // 2, ncc % 2
                    v3 = vau2[t][:, o, :].rearrange("p (h e) -> p h e", e=80)
                    nc.vector.tensor_scalar_mul(
                        v3[:, :, 0:64],
                        ps[:].rearrange("p (h d) -> p h d", d=64),
                        cv_c[:, 16 + ncc:17 + ncc])

            def emit_qt(oc):
                ps = dps_tile()
                for t2 in range(2):
                    nc.tensor.matmul(ps[:],
                                     lhsT=wq8_sb[:, 2 * t2:2 * t2 + 2, ts(oc, 128)],
                                     rhs=xq8_sb[:, 2 * t2:2 * t2 + 2, :],
                                     start=(t2 == 0), stop=(t2 == 1), perf_mode=DR)
                nc.vector.tensor_copy(qt_sb[:, oc, :], ps[:])

            def emit_kt(oc):
                for nn in range(2):
                    ps = dps_tile()
                    for t2 in range(2):
                        nc.tensor.matmul(
                            ps[:],
                            lhsT=wk8_sb[:, 2 * t2:2 * t2 + 2, ts(oc, 128)],
                            rhs=xk8_sb[:, 2 * t2:2 * t2 + 2, ts(nn, 512)],
                            start=(t2 == 0), stop=(t2 == 1), perf_mode=DR)
                    nc.vector.tensor_copy(kt_sb[:, oc, ts(nn, 512)], ps[:])

            def emit_c0():
                # distT carries -ln(LD) and both masks from the host, so the
                # exp needs no per-partition bias and pairs of chunks merge
                for pj in range(4):
                    e_t = stagep.tile([128, 2, NH], BF16, name="e_t", tag="e")
                    nc.scalar.activation(
                        e_t[:].rearrange("p a b -> p (a b)"),
                        dist_sb[:, 2 * pj:2 * pj + 2, :].rearrange(
                            "p a b -> p (a b)"),
                        AF.Exp, bias=0.0, scale=-1.0)
                    for u in range(2):
                        jc = 2 * pj + u
                        nc.vector.scalar_tensor_tensor(
                            out=c0T[:, jc, :], in0=adj_sb[:, jc, :],
                            scalar=cv_c[:, 8 + jc:9 + jc],
                            in1=e_t[:, u, :], op0=OP.mult, op1=OP.add)

            p2 = {}          # (hp, side, t) -> [128, 2, 512] fp8
            pv_ps = {}       # (hp, side) -> [65, 512] psum
            cvs = [None] * 4

            def emit_quad(hp, t):
                ptA = p2p.tile([128, 2, 512], F8D, name="p2", tag="p2")
                ptB = p2p.tile([128, 2, 512], F8D, name="p2", tag="p2")
                p2[(hp, 0, t)] = ptA
                p2[(hp, 1, t)] = ptB
                for u in range(2):
                    jc = 2 * t + u
                    dA = dps_tile()
                    dB = dps_tile()
                    nc.tensor.matmul(dA[:],
                                     lhsT=kt_sb[0:64, hp, ts(jc, 128)],
                                     rhs=qt_sb[0:64, hp, :], start=True,
                                     stop=True, tile_position=(0, 0))
                    nc.tensor.matmul(dB[:],
                                     lhsT=kt_sb[64:128, hp, ts(jc, 128)],
                                     rhs=qt_sb[64:128, hp, :], start=True,
                                     stop=True, tile_position=(64, 0))
                    if (hp, 0, t) in SCHRAUD:
                        for pt, dd in ((ptA, dA), (ptB, dB)):
                            it = i32p.tile([128, 512], I32, name="i32t", tag="i")
                            nc.vector.tensor_scalar(
                                it[:], dd[:], SCH_A * SCALE / 256.0, SCH_B,
                                op0=OP.mult, op1=OP.add)
                            nc.vector.tensor_copy(pt[:, u, :], it[:].bitcast(F32))
                    else:
                        nc.scalar.activation(ptA[:, u, :], dA[:], AF.Exp,
                                             bias=0.0, scale=SCALE / 256.0)
                        nc.scalar.activation(ptB[:, u, :], dB[:], AF.Exp,
                                             bias=0.0, scale=SCALE / 256.0)

            def emit_pv(hp):
                for side in range(2):
                    h = 2 * hp + side
                    pv = pvp.tile([65, NH], F32, name="pv", tag="pv")
                    pv_ps[(hp, side)] = pv
                    for t in range(4):
                        nc.tensor.matmul(pv[:],
                                         lhsT=vau2[t][:, :, ts(h, 80)][:, :, 0:65],
                                         rhs=p2[(hp, side, t)][:],
                                         start=(t == 0), stop=(t == 3), perf_mode=DR)

            pv_sb = {}

            def emit_pv_copy(hp):
                # free the pv psum banks ASAP; kick off the shift/denom DMAs
                sb0 = smlp.tile([65, NH], BF16, name="pvsb0", tag="pvsb0")
                sb1 = smlp.tile([65, NH], BF16, name="pvsb1", tag="pvsb1")
                nc.vector.tensor_copy(sb0[:], pv_ps[(hp, 0)][:])
                nc.vector.tensor_copy(sb1[:], pv_ps[(hp, 1)][:])
                pvs = smlp.tile([128, NH], BF16, name="pvs", tag="pvs")
                nc.gpsimd.dma_start(pvs[64:128, :], sb1[0:64, :])
                r2 = smlp.tile([2, NH], BF16, name="r2", tag="r2")
                nc.gpsimd.dma_start(r2[0:1, :], sb0[64:65, :])
                nc.gpsimd.dma_start(r2[1:2, :], sb1[64:65, :])
                pv_sb[hp] = (sb0, sb1, pvs, r2)

            def emit_norm(hp):
                # denoms -> approx-recip -> *mi -> broadcast -> mul -> +cvs
                sb0, sb1, pvs, r2 = pv_sb[hp]
                r2f = smlp.tile([2, NH], F32, name="r2f", tag="r2f")
                nc.vector.tensor_copy(r2f[:], r2[:])
                rec = smlp.tile([2, NH], F32, name="rec", tag="rec")
                nc.vector.reciprocal_approx_fast(out=rec[:], in_=r2f[:])
                r2m = smlp.tile([2, NH], BF16, name="r2m", tag="r2m")
                nc.vector.tensor_mul(r2m[:], rec[:], mi2_sb[:])
                if hp >= 2:
                    # tail pairs: rank-2 PE outer into a freed dps bank
                    sps_ps = dps_tile()
                    nc.tensor.matmul(sps_ps[:], lhsT=bias2_sb[:, NH:NH + 128],
                                     rhs=r2m[:], start=True, stop=True)
                    s0, s1 = sps_ps[0:64, :], sps_ps[64:128, :]
                else:
                    r2m1 = smlp.tile([1, NH], BF16, name="r2m1", tag="r2m1")
                    nc.gpsimd.dma_start(r2m1[:], r2m[1:2, :])
                    sps = smlp.tile([128, NH], BF16, name="sps", tag="sps")
                    sps1 = smlp.tile([64, NH], BF16, name="sps1", tag="sps1")
                    nc.gpsimd.partition_broadcast(sps[0:64, :], r2m[0:1, :])
                    nc.gpsimd.partition_broadcast(sps1[:], r2m1[0:1, :])
                    nc.gpsimd.dma_start(sps[64:128, :], sps1[:])
                    s0, s1 = sps[0:64, :], sps[64:128, :]
                tmp = smlp.tile([128, NH], BF16, name="tmpb", tag="tmpb")
                nc.vector.tensor_mul(tmp[0:64, :], sb0[0:64, :], s0)
                nc.vector.tensor_mul(tmp[64:128, :], pvs[64:128, :], s1)
                nc.vector.tensor_add(itf_all[:, hp, :], tmp[:], cvs[hp][:])

            def emit_cv(c2):
                cvt = cvp.tile([128, NH], F32, name="cv", tag="cv")
                cvs[c2] = cvt
                for jc in range(8):
                    nc.tensor.matmul(cvt[:], lhsT=v16[:, jc, ts(c2, 128)],
                                     rhs=c0T[:, jc, :], start=(jc == 0),
                                     stop=(jc == 7))

            # ---------------- pipelined emission ----------------
            emit_qt(0)
            emit_kt(0)
            # hp0: dots + q/k(1) + v(0,1); C0 after hp0's ACTs
            for t in range(4):
                emit_quad(0, t)
                if t == 0:
                    emit_qt(1)
                elif t == 1:
                    emit_kt(1)
                elif t == 2:
                    emit_v(0)
                else:
                    emit_v(1)
            emit_c0()
            # hp1: dots + v(2,3) + q/k(2) + cv(0); pv(0) at end
            for t in range(4):
                emit_quad(1, t)
                if t == 0:
                    emit_v(2)
                elif t == 1:
                    emit_v(3)
                elif t == 2:
                    emit_qt(2)
                    emit_kt(2)
                else:
                    emit_cv(0)
            emit_pv(0)
            emit_pv_copy(0)
            # hp2
            for t in range(4):
                emit_quad(2, t)
                if t == 0:
                    emit_qt(3)
                elif t == 1:
                    emit_kt(3)
                elif t == 2:
                    emit_cv(1)
            emit_pv(1)
            emit_pv_copy(1)
            emit_norm(0)
            # hp3
            emit_norm(1)
            for t in range(4):
                emit_quad(3, t)
                if t == 1:
                    emit_cv(2)
                elif t == 2:
                    emit_pv(2)
                    emit_pv_copy(2)
                elif t == 3:
                    emit_norm(2)
            # tail
            emit_pv(3)
            emit_pv_copy(3)
            emit_cv(3)
            emit_norm(3)

            # final out-proj: 4 psum groups on freed dps banks; itf[3] last
            ops = [dps_tile() for _ in range(4)]
            for ic in range(4):
                nc.tensor.matmul(ops[ic][:], lhsT=bias2_sb[:, ts(ic, 128)],
                                 rhs=b2r_sb[:], start=True, stop=False)
                for c2 in range(3):
                    nc.tensor.matmul(ops[ic][:], lhsT=itf_all[:, c2, ts(ic, 128)],
                                     rhs=wout_sb[:, c2, :], start=False, stop=False)
            for ic in range(4):
                nc.tensor.matmul(ops[ic][:], lhsT=itf_all[:, 3, ts(ic, 128)],
                                 rhs=wout_sb[:, 3, :], start=False, stop=True)
                osb = outp.tile([128, D], BF16, name="osb", tag="osb")
                nc.vector.tensor_copy(osb[:], ops[ic][:])
                nc.sync.dma_start(out[ts(ic, 128), :], osb[:])

    nc.compile()
    return nc


def get_nc():
    if "nc" not in _CACHE:
        _CACHE["nc"] = _build_nc()
    return _CACHE["nc"]


def make_in_maps(x, mask, adjacency_mat, distance_mat, W_qkv, W_out, b_out):
    x = np.ascontiguousarray(np.asarray(x, np.float32))
    mask = np.asarray(mask)
    adjacency_mat = np.asarray(adjacency_mat, np.float32)
    distance_mat = np.asarray(distance_mat, np.float32)
    W_qkv = np.asarray(W_qkv, np.float32)
    W_out16 = np.ascontiguousarray(np.asarray(W_out, np.float32)).astype(BF)
    b_out = np.asarray(b_out, np.float32)

    W3 = W_qkv.reshape(D, HEADS, 3, DH)
    wq8 = np.ascontiguousarray(W3[:, :, 0, :].reshape(D, D) * WSC).astype(F8)
    wk8 = np.ascontiguousarray(W3[:, :, 1, :].reshape(D, D) * WSC).astype(F8)
    wv_f32 = np.ascontiguousarray(W3[:, :, 2, :].reshape(D, D))
    wv16 = wv_f32.astype(BF)

    in_maps = []
    for core in range(NCORES):
        b, half = core // 2, core % 2
        i0 = half * NH
        mj = mask[b].astype(np.float32)
        mi = mask[b, i0:i0 + NH].astype(np.float32)

        xTb = np.ascontiguousarray(x[b].T)
        urw = (LA / N) * ((x[b].sum(0) @ wv_f32.astype(np.float32))
                          @ W_out.astype(np.float32))

        cvec = np.zeros((128, 24), np.float32)
        biasj = np.where(mj > 0, np.float32(np.log(LD)), np.float32(NEG))
        cvec[:, 0:8] = biasj.reshape(8, 128).T
        cvec[:, 8:16] = (LG * mj).reshape(8, 128).T
        cvec[:, 16:24] = mj.reshape(8, 128).T

        mjq = np.broadcast_to((mj.reshape(8, 128).T * Q8)[:, :, None],
                              (128, 8, 8))

        adjT = (adjacency_mat[b, i0:i0 + NH, :] * mi[:, None]).T
        # -ln(LD) and both masks folded in: e = exp(-distT) = LD*exp(-d) masked
        distTv = np.where((mi[None, :] > 0) & (mj[:, None] > 0),
                          distance_mat[b].T[:, i0:i0 + NH] -
                          np.float32(np.log(LD)), np.float32(100.0))

        bias2 = np.zeros((2, NH + 128), np.float32)
        bias2[0, :NH] = 1.0
        bias2[1, :NH] = 1.0 - mi
        bias2[0, NH:NH + 64] = 1.0      # E2 row 0 -> sps partitions 0-63
        bias2[1, NH + 64:NH + 128] = 1.0
        b2rv = np.zeros((2, D), np.float32)
        b2rv[0] = b_out
        b2rv[1] = urw

        def arr4(a):
            # [D, X] -> [128, 4, X] with row d = c*128+p at [p, c]
            return np.ascontiguousarray(
                np.asarray(a).reshape(4, 128, -1).transpose(1, 0, 2))

        in_maps.append({
            "xT": arr4(xTb.astype(BF)),
            "xq8": arr4(np.ascontiguousarray(x[b, i0:i0 + NH, :].T).astype(F8)),
            "xk8": arr4(xTb.astype(F8)),
            "wq8": arr4(wq8), "wk8": arr4(wk8), "wv": wv16,
            "wout": arr4(W_out16),
            "adjT8": np.ascontiguousarray(adjT).astype(F8),
            "distT": np.ascontiguousarray(distTv).astype(BF),
            "cvec": cvec,
            "mi2": np.tile((mi * CORR)[None, :], (2, 1)).astype(np.float32),
            "mjq8": np.ascontiguousarray(mjq).astype(F8),
            "bias2": bias2.astype(BF),
            "b2r": b2rv.astype(BF),
        })
    return in_maps


def kernel(x, mask, adjacency_mat, distance_mat, W_qkv, W_out, b_out):
    from concourse.bass_utils import run_bass_kernel_spmd

    nc = get_nc()
    in_maps = make_in_maps(x, mask, adjacency_mat, distance_mat, W_qkv, W_out, b_out)
    res = run_bass_kernel_spmd(nc, in_maps, core_ids=list(range(NCORES)))
    out_full = np.zeros((B, N, D), np.float32)
    for core in range(NCORES):
        b, half = core // 2, core % 2
        out_full[b, half * NH:(half + 1) * NH, :] = \
            np.asarray(res.results[core]["out"]).astype(np.float32)
    return out_full


# revision 41
# speedup vs baseline: 1.1786x; 1.1786x over previous
"""Trainium2 Bass kernel for nn_Attention_3315714753146 (gnn_message_passing).

out = (LA*softmax(mask(QK^T*scale)) + LG*adj_masked + LD*exp(-dist_masked)) @ V @ W_out + b_out

Sharding: 8 shards = (4 batches) x (2 query-row halves of 512 rows). Each core
computes its own 512 output rows from full K/V (computed on-device from x).

Key structure (per core):
  - Softmax path in fp8: q/k projections via fp8 DoubleRow matmuls (weights
    pre-scaled x16 on host), dots as fp8 matmuls, p = exp(dots*scale/256) in
    fp8 ([128,1024] two-bank ACT reads), PV via fp8 DoubleRow with an
    augmented mj/LA column computing the softmax denominator. Softmax term is
    ~0.3% of output norm, so fp8 noise here is invisible at the 2e-2 gate.
  - C0 = LG*adj + LD*exp(-dist) path in bf16: host supplies TRANSPOSED
    masked adjacency (fp8, exact 0/1) and distance (bf16) so no PE transposes
    are needed; cv = V^T @ C0T in bf16 (precision-dominant path).
  - b_out and the invalid-query-row correction (LA/N * colsum(V) @ W_out)
    enter as one K=2 rank-2 matmul per output row-chunk.
  - Emission is software-pipelined across head-pairs to keep the PE stream
    dense (HAM stays warm) and start the Scalar exp stream by ~4us.
"""

import sys

for _p in ("/root/.axon_site", "/root/.axon_site/_ro/trn_rl_repo",
           "/root/.axon_site/_ro/pypackages"):
    if _p not in sys.path:
        sys.path.append(_p)

import numpy as np
import ml_dtypes

BF = ml_dtypes.bfloat16
F8 = ml_dtypes.float8_e4m3
HEADS, DH = 8, 64
B, N, D = 4, 1024, 512
NH = 512          # query rows per core
LA = LD = LG = 0.33
SCALE = DH ** -0.5
NEG = -1e30
NCORES = 8
WSC = 16.0        # host prescale on wq/wk; dots come out x256
Q8 = float(np.float32(F8(1.0 / LA)))   # fp8 value used in denominator column
CORR = LA * Q8
# Schraudolph exp constants: exp(z) ~ bitcast_f32(int32(A*z + Bc))
SCH_A = 8388608.0 / float(np.log(2.0))
SCH_B = 127.0 * 8388608.0 - 366000.0
# which (hp, side, t) exp tiles run on DVE instead of Scalar
SCHRAUD = set()

_CACHE = {}


def _build_nc():
    import concourse.bass as bass
    import concourse.bacc as bacc
    import concourse.tile as tile
    from concourse import mybir
    from concourse.bass import ts

    F32 = mybir.dt.float32
    BF16 = mybir.dt.bfloat16
    F8D = mybir.dt.float8e4
    I32 = mybir.dt.int32
    AF = mybir.ActivationFunctionType
    OP = mybir.AluOpType
    DR = mybir.MatmulPerfMode.DoubleRow

    nc = bacc.Bacc()
    xT = nc.declare_dram_parameter("xT", [128, 4, N], BF16, isOutput=False)
    xq8 = nc.declare_dram_parameter("xq8", [128, 4, NH], F8D, isOutput=False)
    xk8 = nc.declare_dram_parameter("xk8", [128, 4, N], F8D, isOutput=False)
    wq8 = nc.declare_dram_parameter("wq8", [128, 4, D], F8D, isOutput=False)
    wk8 = nc.declare_dram_parameter("wk8", [128, 4, D], F8D, isOutput=False)
    wv = nc.declare_dram_parameter("wv", [D, D], BF16, isOutput=False)
    wout = nc.declare_dram_parameter("wout", [128, 4, D], BF16, isOutput=False)
    adjT8 = nc.declare_dram_parameter("adjT8", [N, NH], F8D, isOutput=False)
    distT = nc.declare_dram_parameter("distT", [N, NH], BF16, isOutput=False)
    # cvec cols: [0:8]=lnLD+ln(mj) per j-chunk, [8:16]=LG*mj, [16:24]=mj
    cvec = nc.declare_dram_parameter("cvec", [128, 24], F32, isOutput=False)
    mi2 = nc.declare_dram_parameter("mi2", [2, NH], F32, isOutput=False)
    mjq8 = nc.declare_dram_parameter("mjq8", [128, 8, 8], F8D, isOutput=False)
    bias2 = nc.declare_dram_parameter("bias2", [2, NH + 128], BF16, isOutput=False)
    b2r = nc.declare_dram_parameter("b2r", [2, D], BF16, isOutput=False)
    out = nc.declare_dram_parameter("out", [NH, D], BF16, isOutput=True)

    with tile.TileContext(nc) as tc:
        with (
            tc.tile_pool(name="const", bufs=1) as constp,
            tc.tile_pool(name="pers", bufs=1) as pers,
            tc.tile_pool(name="stage", bufs=2) as stagep,
            tc.tile_pool(name="p2p", bufs=16) as p2p,
            tc.tile_pool(name="sml", bufs=4) as smlp,
            tc.tile_pool(name="i32p", bufs=2) as i32p,
            tc.tile_pool(name="outp", bufs=2) as outp,
            tc.tile_pool(name="dpsp", bufs=5, space="PSUM") as dpsp,
            tc.tile_pool(name="pvp", bufs=2, space="PSUM") as pvp,
            tc.tile_pool(name="cvp", bufs=1, space="PSUM") as cvp,
        ):
            # ---------------- DMAs (priority order: q/k first) -------------
            xq8_sb = pers.tile([128, 4, NH], F8D, name="xq8_sb")
            nc.sync.dma_start(xq8_sb[:], xq8[:])
            wq8_sb = pers.tile([128, 4, D], F8D, name="wq8_sb")
            nc.sync.dma_start(wq8_sb[:], wq8[:])
            wk8_sb = pers.tile([128, 4, D], F8D, name="wk8_sb")
            nc.sync.dma_start(wk8_sb[:], wk8[:])
            xk8_sb = pers.tile([128, 4, N], F8D, name="xk8_sb")
            nc.sync.dma_start(xk8_sb[:], xk8[:])
            wv_sb = pers.tile([128, 4, D], BF16, name="wv_sb")
            for cc in range(4):
                nc.sync.dma_start(wv_sb[:, cc, :], wv[ts(cc, 128), :])
            xt = pers.tile([128, 4, N], BF16, name="xt")
            for ncc in range(8):
                nc.sync.dma_start(xt[:, :, ts(ncc, 128)], xT[:, :, ts(ncc, 128)])
            adj_sb = pers.tile([128, 8, NH], F8D, name="adj_sb")
            dist_sb = pers.tile([128, 8, NH], BF16, name="dist_sb")
            for jc in range(8):
                nc.sync.dma_start(dist_sb[:, jc, :], distT[ts(jc, 128), :])
            for jc in range(8):
                nc.sync.dma_start(adj_sb[:, jc, :], adjT8[ts(jc, 128), :])
            wout_sb = pers.tile([128, 4, D], BF16, name="wout_sb")
            nc.sync.dma_start(wout_sb[:], wout[:])
            cv_c = constp.tile([128, 24], F32, name="cv_c")
            nc.gpsimd.dma_start(cv_c[:], cvec[:])
            mi2_sb = constp.tile([2, NH], F32, name="mi2_sb")
            nc.gpsimd.dma_start(mi2_sb[:], mi2[:])
            bias2_sb = constp.tile([2, NH + 128], BF16, name="bias2_sb")
            nc.gpsimd.dma_start(bias2_sb[:], bias2[:])
            b2r_sb = constp.tile([2, D], BF16, name="b2r_sb")
            nc.gpsimd.dma_start(b2r_sb[:], b2r[:])
            mjq_sb = constp.tile([128, 8, 8], F8D, name="mjq_sb")
            nc.gpsimd.dma_start(mjq_sb[:], mjq8[:])

            # vau2[t]: [128, o, 8 heads * 80] fp8; cols h*80..h*80+63 = v*mj,
            # col h*80+64 = mj/LA, rest padding
            vau2 = [pers.tile([128, 2, 640], F8D, name=f"vau2_{t}") for t in range(4)]
            for t in range(4):
                for o in range(2):
                    v3 = vau2[t][:, o, :].rearrange("p (h e) -> p h e", e=80)
                    nc.gpsimd.tensor_copy(v3[:, :, 64], mjq_sb[:, 2 * t + o, :])

            v16 = pers.tile([128, 8, NH], BF16, name="v16")
            qt_sb = pers.tile([128, 4, NH], F8D, name="qt_sb")
            kt_sb = pers.tile([128, 4, N], F8D, name="kt_sb")
            c0T = pers.tile([128, 8, NH], BF16, name="c0T")
            itf_all = pers.tile([128, 4, NH], BF16, name="itf_all")

            def dps_tile():
                return dpsp.tile([128, 512], F32, name="dps", tag="dps")

            def emit_v(pr):
                for half in range(2):
                    ncc = 2 * pr + half
                    ps = dps_tile()
                    for cc in range(4):
                        nc.tensor.matmul(ps[:], lhsT=xt[:, cc, ts(ncc, 128)],
                                         rhs=wv_sb[:, cc, :], start=(cc == 0),
                                         stop=(cc == 3))
                    nc.vector.tensor_copy(v16[:, ncc, :], ps[:])
                    t, o = ncc // 2, ncc % 2
                    v3 = vau2[t][:, o, :].rearrange("p (h e) -> p h e", e=80)
                    nc.vector.tensor_scalar_mul(
                        v3[:, :, 0:64],
                        ps[:].rearrange("p (h d) -> p h d", d=64),
                        cv_c[:, 16 + ncc:17 + ncc])

            def emit_qt(oc):
                ps = dps_tile()
                for t2 in range(2):
                    nc.tensor.matmul(ps[:],
                                     lhsT=wq8_sb[:, 2 * t2:2 * t2 + 2, ts(oc, 128)],
                                     rhs=xq8_sb[:, 2 * t2:2 * t2 + 2, :],
                                     start=(t2 == 0), stop=(t2 == 1), perf_mode=DR)
                nc.vector.tensor_copy(qt_sb[:, oc, :], ps[:])

            def emit_kt(oc):
                for nn in range(2):
                    ps = dps_tile()
                    for t2 in range(2):
                        nc.tensor.matmul(
                            ps[:],
                            lhsT=wk8_sb[:, 2 * t2:2 * t2 + 2, ts(oc, 128)],
                            rhs=xk8_sb[:, 2 * t2:2 * t2 + 2, ts(nn, 512)],
                            start=(t2 == 0), stop=(t2 == 1), perf_mode=DR)
                    nc.vector.tensor_copy(kt_sb[:, oc, ts(nn, 512)], ps[:])

            def emit_c0():
                # distT carries -ln(LD) and both masks from the host, so the
                # exp needs no per-partition bias and pairs of chunks merge
                for pj in range(4):
                    e_t = stagep.tile([128, 2, NH], BF16, name="e_t", tag="e")
                    nc.scalar.activation(
                        e_t[:].rearrange("p a b -> p (a b)"),
                        dist_sb[:, 2 * pj:2 * pj + 2, :].rearrange(
                            "p a b -> p (a b)"),
                        AF.Exp, bias=0.0, scale=-1.0)
                    for u in range(2):
                        jc = 2 * pj + u
                        nc.vector.scalar_tensor_tensor(
                            out=c0T[:, jc, :], in0=adj_sb[:, jc, :],
                            scalar=cv_c[:, 8 + jc:9 + jc],
                            in1=e_t[:, u, :], op0=OP.mult, op1=OP.add)

            p2 = {}          # (hp, side, t) -> [128, 2, 512] fp8
            pv_ps = {}       # (hp, side) -> [65, 512] psum
            cvs = [None] * 4

            def emit_quad(hp, t):
                ptA = p2p.tile([128, 2, 512], F8D, name="p2", tag="p2")
                ptB = p2p.tile([128, 2, 512], F8D, name="p2", tag="p2")
                p2[(hp, 0, t)] = ptA
                p2[(hp, 1, t)] = ptB
                for u in range(2):
                    jc = 2 * t + u
                    dA = dps_tile()
                    dB = dps_tile()
                    nc.tensor.matmul(dA[:],
                                     lhsT=kt_sb[0:64, hp, ts(jc, 128)],
                                     rhs=qt_sb[0:64, hp, :], start=True,
                                     stop=True, tile_position=(0, 0))
                    nc.tensor.matmul(dB[:],
                                     lhsT=kt_sb[64:128, hp, ts(jc, 128)],
                                     rhs=qt_sb[64:128, hp, :], start=True,
                                     stop=True, tile_position=(64, 0))
                    if (hp, 0, t) in SCHRAUD:
                        for pt, dd in ((ptA, dA), (ptB, dB)):
                            it = i32p.tile([128, 512], I32, name="i32t", tag="i")
                            nc.vector.tensor_scalar(
                                it[:], dd[:], SCH_A * SCALE / 256.0, SCH_B,
                                op0=OP.mult, op1=OP.add)
                            nc.vector.tensor_copy(pt[:, u, :], it[:].bitcast(F32))
                    else:
                        nc.scalar.activation(ptA[:, u, :], dA[:], AF.Exp,
                                             bias=0.0, scale=SCALE / 256.0)
                        nc.scalar.activation(ptB[:, u, :], dB[:], AF.Exp,
                                             bias=0.0, scale=SCALE / 256.0)

            def emit_pv(hp):
                for side in range(2):
                    h = 2 * hp + side
                    pv = pvp.tile([65, NH], F32, name="pv", tag="pv")
                    pv_ps[(hp, side)] = pv
                    for t in range(4):
                        nc.tensor.matmul(pv[:],
                                         lhsT=vau2[t][:, :, ts(h, 80)][:, :, 0:65],
                                         rhs=p2[(hp, side, t)][:],
                                         start=(t == 0), stop=(t == 3), perf_mode=DR)

            pv_sb = {}

            def emit_pv_copy(hp):
                # free the pv psum banks ASAP; kick off the shift/denom DMAs
                sb0 = smlp.tile([65, NH], BF16, name="pvsb0", tag="pvsb0")
                sb1 = smlp.tile([65, NH], BF16, name="pvsb1", tag="pvsb1")
                nc.vector.tensor_copy(sb0[:], pv_ps[(hp, 0)][:])
                nc.vector.tensor_copy(sb1[:], pv_ps[(hp, 1)][:])
                pvs = smlp.tile([128, NH], BF16, name="pvs", tag="pvs")
                nc.gpsimd.dma_start(pvs[64:128, :], sb1[0:64, :])
                r2 = smlp.tile([2, NH], BF16, name="r2", tag="r2")
                nc.gpsimd.dma_start(r2[0:1, :], sb0[64:65, :])
                nc.gpsimd.dma_start(r2[1:2, :], sb1[64:65, :])
                pv_sb[hp] = (sb0, sb1, pvs, r2)

            def emit_norm(hp):
                # denoms -> approx-recip -> *mi -> broadcast -> mul -> +cvs
                sb0, sb1, pvs, r2 = pv_sb[hp]
                r2f = smlp.tile([2, NH], F32, name="r2f", tag="r2f")
                nc.vector.tensor_copy(r2f[:], r2[:])
                rec = smlp.tile([2, NH], F32, name="rec", tag="rec")
                nc.vector.reciprocal_approx_fast(out=rec[:], in_=r2f[:])
                r2m = smlp.tile([2, NH], BF16, name="r2m", tag="r2m")
                nc.vector.tensor_mul(r2m[:], rec[:], mi2_sb[:])
                if hp >= 2:
                    # tail pairs: rank-2 PE outer into a freed dps bank
                    sps_ps = dps_tile()
                    nc.tensor.matmul(sps_ps[:], lhsT=bias2_sb[:, NH:NH + 128],
                                     rhs=r2m[:], start=True, stop=True)
                    s0, s1 = sps_ps[0:64, :], sps_ps[64:128, :]
                else:
                    r2m1 = smlp.tile([1, NH], BF16, name="r2m1", tag="r2m1")
                    nc.gpsimd.dma_start(r2m1[:], r2m[1:2, :])
                    sps = smlp.tile([128, NH], BF16, name="sps", tag="sps")
                    sps1 = smlp.tile([64, NH], BF16, name="sps1", tag="sps1")
                    nc.gpsimd.partition_broadcast(sps[0:64, :], r2m[0:1, :])
                    nc.gpsimd.partition_broadcast(sps1[:], r2m1[0:1, :])
                    nc.gpsimd.dma_start(sps[64:128, :], sps1[:])
                    s0, s1 = sps[0:64, :], sps[64:128, :]
                tmp = smlp.tile([128, NH], BF16, name="tmpb", tag="tmpb")
                nc.vector.tensor_mul(tmp[0:64, :], sb0[0:64, :], s0)
                nc.vector.tensor_mul(tmp[64:128, :], pvs[64:128, :], s1)
                nc.vector.tensor_add(itf_all[:, hp, :], tmp[:], cvs[hp][:])

            def emit_cv(c2):
                cvt = cvp.tile([128, NH], F32, name="cv", tag="cv")
                cvs[c2] = cvt
                for jc in range(8):
                    nc.tensor.matmul(cvt[:], lhsT=v16[:, jc, ts(c2, 128)],
                                     rhs=c0T[:, jc, :], start=(jc == 0),
                                     stop=(jc == 7))

            # ---------------- pipelined emission ----------------
            emit_qt(0)
            emit_kt(0)
            # hp0: dots + q/k(1) + v(0,1); C0 after hp0's ACTs
            for t in range(4):
                emit_quad(0, t)
                if t == 0:
                    emit_qt(1)
                elif t == 1:
                    emit_kt(1)
                elif t == 2:
                    emit_v(0)
                else:
                    emit_v(1)
            emit_c0()
            # hp1: dots + v(2,3) + q/k(2) + cv(0); pv(0) at end
            for t in range(4):
                emit_quad(1, t)
                if t == 0:
                    emit_v(2)
                elif t == 1:
                    emit_v(3)
                elif t == 2:
                    emit_qt(2)
                    emit_kt(2)
                else:
                    emit_cv(0)
            emit_pv(0)
            emit_pv_copy(0)
            # hp2
            for t in range(4):
                emit_quad(2, t)
                if t == 0:
                    emit_qt(3)
                elif t == 1:
                    emit_kt(3)
                elif t == 2:
                    emit_cv(1)
            emit_pv(1)
            emit_pv_copy(1)
            emit_norm(0)
            # hp3
            emit_norm(1)
            for t in range(4):
                emit_quad(3, t)
                if t == 1:
                    emit_cv(2)
                elif t == 2:
                    emit_pv(2)
                    emit_pv_copy(2)
                elif t == 3:
                    emit_norm(2)
            # tail
            emit_pv(3)
            emit_pv_copy(3)
            emit_cv(3)
            emit_norm(3)

            # final out-proj: 4 psum groups on freed dps banks; itf[3] last
            ops = [dps_tile() for _ in range(4)]
            for ic in range(4):
                nc.tensor.matmul(ops[ic][:], lhsT=bias2_sb[:, ts(ic, 128)],
                                 rhs=b2r_sb[:], start=True, stop=False)
                for c2 in range(3):
                    nc.tensor.matmul(ops[ic][:], lhsT=itf_all[:, c2, ts(ic, 128)],
                                     rhs=wout_sb[:, c2, :], start=False, stop=False)
            for ic in range(4):
                nc.tensor.matmul(ops[ic][:], lhsT=itf_all[:, 3, ts(ic, 128)],
                                 rhs=wout_sb[:, 3, :], start=False, stop=True)
                osb = outp.tile([128, D], BF16, name="osb", tag="osb")
                nc.vector.tensor_copy(osb[:], ops[ic][:])
                nc.sync.dma_start(out[ts(ic, 128), :], osb[:])

    nc.compile()
    return nc


def get_nc():
    if "nc" not in _CACHE:
        _CACHE["nc"] = _build_nc()
    return _CACHE["nc"]


def make_in_maps(x, mask, adjacency_mat, distance_mat, W_qkv, W_out, b_out):
    x = np.ascontiguousarray(np.asarray(x, np.float32))
    mask = np.asarray(mask)
    adjacency_mat = np.asarray(adjacency_mat, np.float32)
    distance_mat = np.asarray(distance_mat, np.float32)
    W_qkv = np.asarray(W_qkv, np.float32)
    W_out16 = np.ascontiguousarray(np.asarray(W_out, np.float32)).astype(BF)
    b_out = np.asarray(b_out, np.float32)

    W3 = W_qkv.reshape(D, HEADS, 3, DH)
    wq8 = np.ascontiguousarray(W3[:, :, 0, :].reshape(D, D) * WSC).astype(F8)
    wk8 = np.ascontiguousarray(W3[:, :, 1, :].reshape(D, D) * WSC).astype(F8)
    wv_f32 = np.ascontiguousarray(W3[:, :, 2, :].reshape(D, D))
    wv16 = wv_f32.astype(BF)

    in_maps = []
    for core in range(NCORES):
        b, half = core // 2, core % 2
        i0 = half * NH
        mj = mask[b].astype(np.float32)
        mi = mask[b, i0:i0 + NH].astype(np.float32)

        xTb = np.ascontiguousarray(x[b].T)
        urw = (LA / N) * ((x[b].sum(0) @ wv_f32.astype(np.float32))
                          @ W_out.astype(np.float32))

        cvec = np.zeros((128, 24), np.float32)
        biasj = np.where(mj > 0, np.float32(np.log(LD)), np.float32(NEG))
        cvec[:, 0:8] = biasj.reshape(8, 128).T
        cvec[:, 8:16] = (LG * mj).reshape(8, 128).T
        cvec[:, 16:24] = mj.reshape(8, 128).T

        mjq = np.broadcast_to((mj.reshape(8, 128).T * Q8)[:, :, None],
                              (128, 8, 8))

        adjT = (adjacency_mat[b, i0:i0 + NH, :] * mi[:, None]).T
        # -ln(LD) and both masks folded in: e = exp(-distT) = LD*exp(-d) masked
        distTv = np.where((mi[None, :] > 0) & (mj[:, None] > 0),
                          distance_mat[b].T[:, i0:i0 + NH] -
                          np.float32(np.log(LD)), np.float32(100.0))

        bias2 = np.zeros((2, NH + 128), np.float32)
        bias2[0, :NH] = 1.0
        bias2[1, :NH] = 1.0 - mi
        bias2[0, NH:NH + 64] = 1.0      # E2 row 0 -> sps partitions 0-63
        bias2[1, NH + 64:NH + 128] = 1.0
        b2rv = np.zeros((2, D), np.float32)
        b2rv[0] = b_out
        b2rv[1] = urw

        def arr4(a):
            # [D, X] -> [128, 4, X] with row d = c*128+p at [p, c]
            return np.ascontiguousarray(
                np.asarray(a).reshape(4, 128, -1).transpose(1, 0, 2))

        in_maps.append({
            "xT": arr4(xTb.astype(BF)),
            "xq8": arr4(np.ascontiguousarray(x[b, i0:i0 + NH, :].T).astype(F8)),
            "xk8": arr4(xTb.astype(F8)),
            "wq8": arr4(wq8), "wk8": arr4(wk8), "wv": wv16,
            "wout": arr4(W_out16),
            "adjT8": np.ascontiguousarray(adjT).astype(F8),
            "distT": np.ascontiguousarray(distTv).astype(BF),
            "cvec": cvec,
            "mi2": np.tile((mi * CORR)[None, :], (2, 1)).astype(np.float32),
            "mjq8": np.ascontiguousarray(mjq).astype(F8),
            "bias2": bias2.astype(BF),
            "b2r": b2rv.astype(BF),
        })
    return in_maps


def kernel(x, mask, adjacency_mat, distance_mat, W_qkv, W_out, b_out):
    from concourse.bass_utils import run_bass_kernel_spmd

    nc = get_nc()
    in_maps = make_in_maps(x, mask, adjacency_mat, distance_mat, W_qkv, W_out, b_out)
    res = run_bass_kernel_spmd(nc, in_maps, core_ids=list(range(NCORES)))
    out_full = np.zeros((B, N, D), np.float32)
    for core in range(NCORES):
        b, half = core // 2, core % 2
        out_full[b, half * NH:(half + 1) * NH, :] = \
            np.asarray(res.results[core]["out"]).astype(np.float32)
    return out_full
